# revision 1
# baseline (speedup 1.0000x reference)
"""Trainium2 Bass kernel for causal self-attention (B=2, S=2048, D=1024, H=16).

Sharding: 8 cores = 2 batches x 4 head-groups. Core c handles batch c//4 and
heads 4*(c%4) .. 4*(c%4)+4. Each core receives its batch's x [2048, 1024] and
its [1024, 768] slice of w_qkv (q/k/v columns for its 4 heads), and produces
the [2048, 256] output slice. No cross-core communication is needed; the host
gathers the slices. w_o is unused by the reference (no output projection).

Per-core kernel (Tile framework), fp16 matmul path with fp32 psum/softmax:
  1. x arrives host-transposed as xT [d, s] (fp16); plain chunked DMAs.
  2. Projection with w as stationary produces qT/kT [cols, s] directly; v is
     produced in natural [s, hd] layout (xT as stationary) and augmented with
     a ones column (vaug) so the AV matmul also emits softmax denominators.
  3. Attention per head pair, per 512-wide query chunk: scores are computed
     transposed ST[j, i] = k_j . q_i with two heads packed into the 128-row
     PE array (K=64 row groups); exp on ACT over [128, 2048] PSUM groups;
     causal mask = affine_select on the diagonal 128x128 block + restricted
     AV column ranges; AV accumulates outT [65, i] with v||ones stationary;
     a final PE transpose + reciprocal*mul yields the natural-layout output.
"""

import sys

sys.path.insert(0, "/opt/trn_rl_repo")

from contextlib import ExitStack

import numpy as np

import concourse.bass as bass
import concourse.tile as tile
from concourse import bacc, masks, mybir
from concourse.bass_utils import run_bass_kernel_spmd

B, S, D, H = 2, 2048, 1024, 16
HD = 64          # head dim
HPC = 4          # heads per core
NCORES = 8
P = 128
NS = S // P      # 16 s-blocks
KC = D // P      # 8 d-chunks
CH = 512         # query-chunk width
NT = S // CH     # 4 query chunks
COLS = 3 * HPC * HD   # 768 projection columns per core
F32 = mybir.dt.float32
F16 = mybir.dt.float16
SCALE = 1.0 / np.sqrt(HD)

PSUM = bass.MemorySpace.PSUM


def _build_body(ctx: ExitStack, tc: "tile.TileContext", x_d, w_d, o_d):
    nc = tc.nc

    persist = ctx.enter_context(tc.tile_pool(name="persist", bufs=1))
    ident_h = persist.tile([P, P], F16)
    masks.make_identity(nc, ident_h[:])

    # v in natural layout + ones column, per (j-block, head): [128, 65] slices
    vaug = persist.tile([P, NS * HPC * 65], F16)
    nc.vector.memset(
        vaug[:].rearrange("p (n c) -> p n c", c=65)[:, :, 64:65], 1.0
    )
    # final output staging [128, 16 i-blocks * 4 heads * 64]
    out_sb = persist.tile([P, NS * HPC * HD], F32)

    wp = ctx.enter_context(tc.tile_pool(name="w", bufs=1))
    xtp = ctx.enter_context(tc.tile_pool(name="xT", bufs=1))
    w_all = wp.tile([P, KC * COLS], F16)     # [128, 8*768]
    for k in range(KC):
        nc.sync.dma_start(
            w_all[:, k * COLS:(k + 1) * COLS],
            w_d[k * P:(k + 1) * P, :],
        )
    xT = xtp.tile([P, KC * S], F16)          # [128, 8*2048]

    # ---- t-pipelined main loop -------------------------------------------
    qkp = ctx.enter_context(tc.tile_pool(name="qk", bufs=1))
    ps_small = ctx.enter_context(tc.tile_pool(name="ps_small", bufs=2, space=PSUM))
    ps_st = ctx.enter_context(tc.tile_pool(name="ps_st", bufs=2, space=PSUM))
    ps_o = ctx.enter_context(tc.tile_pool(name="ps_o", bufs=2, space=PSUM))
    pp = ctx.enter_context(tc.tile_pool(name="p", bufs=4))
    osbp = ctx.enter_context(tc.tile_pool(name="osb", bufs=3))
    rcp = ctx.enter_context(tc.tile_pool(name="rcol", bufs=4))

    qkT = qkp.tile([P, 4 * S], F16)      # m0,m1 = q(h01,h23); m2,m3 = k

    def xtp_group(ig):
        """Load xT column chunks for s-range 512*ig..512*(ig+1).

        x arrives from the host already transposed ([D, S], fp16), so these
        are plain contiguous DMAs."""
        for k in range(KC):
            nc.sync.dma_start(
                xT[:, k * S + ig * 512: k * S + (ig + 1) * 512],
                x_d[k * P:(k + 1) * P, ig * 512:(ig + 1) * 512],
            )

    def proj(m, sblk):
        """qkT[:, m*S + sblk chunk] = (w col-block m)^T @ x^T."""
        pp_ps = ps_small.tile([P, 512], F32, tag="small")
        for k in range(KC):
            nc.tensor.matmul(
                pp_ps[:],
                w_all[:, k * COLS + m * P: k * COLS + (m + 1) * P],
                xT[:, k * S + sblk * 512: k * S + (sblk + 1) * 512],
                start=(k == 0),
                stop=(k == KC - 1),
            )
        nc.vector.tensor_copy(
            qkT[:, m * S + sblk * 512: m * S + (sblk + 1) * 512], pp_ps[:]
        )

    def vdirect(sb):
        """vaug[s-block sb] = x[sb] @ w_v (natural layout), all 4 heads."""
        pv = ps_small.tile([P, 512], F32, tag="small")
        for k in range(KC):
            nc.tensor.matmul(
                pv[:, 0:256],
                xT[:, k * S + sb * P: k * S + (sb + 1) * P],
                w_all[:, k * COLS + 512: k * COLS + 768],
                start=(k == 0),
                stop=(k == KC - 1),
            )
        nc.vector.tensor_copy(
            vaug[:, sb * HPC * 65:(sb + 1) * HPC * 65]
            .rearrange("p (g c) -> p g c", c=65)[:, :, 0:64],
            pv[:, 0:256].rearrange("p (g c) -> p g c", c=64),
        )

    out_view = out_sb[:].rearrange("p (i g d) -> p i g d", g=HPC, d=HD)

    def attn(pair, t):
        """Heads 2*pair, 2*pair+1; query chunk t (i in [512t, 512t+512))."""
        hA, hB = 2 * pair, 2 * pair + 1
        qm, km = pair, 2 + pair
        po_a = ps_o.tile([65, 512], F32, tag="o")
        po_b = ps_o.tile([65, 512], F32, tag="o")
        po = {hA: po_a, hB: po_b}
        njb = 4 * t + 4
        for jb in range(njb):  # one j-block (both heads) per group
            st = ps_st.tile([P, 1024], F32, tag="st")
            # ST[j, i] = k_j . q_i ; heads packed in PE rows 0-63 / 64-127
            for hi, h in enumerate((hA, hB)):
                hb = (h % 2) * 64
                nc.tensor.matmul(
                    st[:, hi * 512:(hi + 1) * 512],
                    qkT[hb:hb + 64, km * S + jb * P: km * S + (jb + 1) * P],
                    qkT[hb:hb + 64, qm * S + t * 512: qm * S + (t + 1) * 512],
                    start=True,
                    stop=True,
                    tile_position=(hb, 0),
                )
            p_t = pp.tile([P, 1024], F16, tag="p")
            nc.scalar.activation(
                p_t[:], st[:], mybir.ActivationFunctionType.Exp,
                scale=float(SCALE),
            )
            doff = jb - 4 * t
            if doff >= 0:
                # triangular mask on the diagonal 128x128 sub-block only;
                # the fully-masked zone is skipped by AV column ranges.
                for hi in range(2):
                    c0 = hi * 512 + 128 * doff
                    sl = p_t[:, c0:c0 + 128]
                    nc.gpsimd.affine_select(
                        out=sl,
                        in_=sl,
                        compare_op=mybir.AluOpType.is_ge,
                        fill=0.0,
                        base=0,
                        channel_multiplier=-1,
                        pattern=[[1, 128]],
                    )
            off = max(0, 128 * doff)
            for hi, h in enumerate((hA, hB)):
                nc.tensor.matmul(
                    po[h][:, off:512],
                    vaug[:, (jb * HPC + h) * 65: (jb * HPC + h + 1) * 65],
                    p_t[:, hi * 512 + off:(hi + 1) * 512],
                    start=(jb == 0),
                    stop=(jb == njb - 1),
                )
        # finalize: transpose outT to natural, divide by denominator
        for h in (hA, hB):
            osb_t = osbp.tile([65, 512], F16, tag="osb")
            nc.vector.tensor_copy(osb_t[:], po[h][:])
            fin32 = ps_small.tile([P, 512], F32, tag="small")
            fin = fin32.bitcast(F16)[:, 0:512]
            for b in range(4):
                nc.tensor.transpose(
                    fin[:, b * P:b * P + 65],
                    osb_t[:, b * P:(b + 1) * P],
                    ident_h[0:65, 0:65],
                )
            fin_view = fin[:, 0:512].rearrange("p (n c) -> p n c", c=P)
            rc = rcp.tile([P, 4], F32, tag="rc")
            nc.vector.reciprocal(rc[:], fin_view[:, :, 64])
            nc.vector.tensor_mul(
                out_view[:, 4 * t:4 * t + 4, h, :],
                fin_view[:, :, 0:64],
                rc[:].broadcast_to([P, 4, HD]),
            )

    for t in range(NT):
        xtp_group(t)
        for m in (0, 2, 1, 3):
            proj(m, t)
        for sb in range(4 * t, 4 * t + 4):
            vdirect(sb)
        attn(0, t)
        attn(1, t)
        for b in range(4):
            ib = 4 * t + b
            nc.sync.dma_start(
                o_d[ib * P:(ib + 1) * P, :],
                out_sb[:, ib * HPC * HD:(ib + 1) * HPC * HD],
            )


def build_program():
    nc = bacc.Bacc(
        "TRN2",
        target_bir_lowering=False,
        debug=False,
        enable_asserts=True,
    )
    x_d = nc.dram_tensor("x", [D, S], F16, kind="ExternalInput").ap()
    w_d = nc.dram_tensor("w", [D, COLS], F16, kind="ExternalInput").ap()
    o_d = nc.dram_tensor("o", [S, HPC * HD], F32, kind="ExternalOutput").ap()

    with tile.TileContext(nc) as tc, ExitStack() as ctx:
        _build_body(ctx, tc, x_d, w_d, o_d)
    nc.compile()
    return nc


_CACHE = {}


def _compiled():
    if "nc" not in _CACHE:
        _CACHE["nc"] = build_program()
    return _CACHE["nc"]


def make_in_maps(x, w_qkv):
    x = np.asarray(x, dtype=np.float32)
    w_qkv = np.asarray(w_qkv, dtype=np.float32)
    # one transpose+cast per batch; cores sharing a batch reuse the array
    xT16 = [x[b].T.astype(np.float16) for b in range(B)]
    in_maps = []
    for c in range(NCORES):
        b = c // 4
        cs = (c % 4) * HPC * HD
        w_slice = np.concatenate(
            [
                w_qkv[:, cs:cs + HPC * HD],
                w_qkv[:, D + cs:D + cs + HPC * HD],
                w_qkv[:, 2 * D + cs:2 * D + cs + HPC * HD],
            ],
            axis=1,
        )
        in_maps.append(
            {
                "x": xT16[b],
                "w": np.ascontiguousarray(w_slice).astype(np.float16),
            }
        )
    return in_maps


def gather_out(results):
    out = np.empty((B, S, D), np.float32)
    for c in range(NCORES):
        b = c // 4
        cs = (c % 4) * HPC * HD
        out[b][:, cs:cs + HPC * HD] = results[c]["o"]
    return out


def kernel(x, w_qkv, w_o=None, **_):
    nc = _compiled()
    res = run_bass_kernel_spmd(nc, make_in_maps(x, w_qkv), core_ids=list(range(NCORES)))
    return gather_out(res.results)



# revision 28
# speedup vs baseline: 1.0166x; 1.0166x over previous
"""Trainium2 Bass kernel for causal self-attention (B=2, S=2048, D=1024, H=16).

Sharding: 8 cores = 2 batches x 4 head-groups. Core c handles batch c//4 and
heads 4*(c%4) .. 4*(c%4)+4. No cross-core communication; the host gathers the
output slices. w_o is unused by the reference.

Per-core kernel (Tile framework), fp16 matmul path with fp32 psum/softmax:
  1. All xT chunks + q/k weights are DMA'd upfront, interleaved per d-chunk
     on the sync queue so the first projection matmul starts ~1.5us in; wv
     goes on the scalar queue. Output DMAs are emitted per i-block but sit
     behind the (already drained) input DMAs, so they never stall the next
     chunk's projection.
  2. Projection with w stationary produces qT/kT [cols, s]; v is produced in
     natural [s, hd] layout (xT stationary) and augmented with a ones column
     (vaug) so the AV matmul also emits softmax denominators.
  3. Scores ST[j,i] = k_j . q_i per j-block with two heads packed into PE
     rows 0-63 / 64-127, trimmed to the causal window. Triangular masking of
     the diagonal 128x128 block happens inside the same PSUM accumulation
     group: one extra matmul adds A^T @ B = -235*max(0, j-i) (A[r,j]=[r<=j]
     stationary, B[r,i]=-235*[r>i] moving), so exp yields exact zeros and no
     other engine sits on the ST->exp critical path.
  4. exp on ACT (bias -3.25, softmax-invariant; true max scaled score ~7.95)
     over both heads in one instruction, split per head only for deep
     diagonal blocks where trimming saves real elements. AV accumulates
     outT[65, i] with v||ones stationary over the trimmed causal column
     range; a final PE transpose + reciprocal*mul yields natural layout.
"""

import sys

sys.path.insert(0, "/opt/trn_rl_repo")

from contextlib import ExitStack

import numpy as np

import concourse.bass as bass
import concourse.tile as tile
from concourse import bacc, masks, mybir
from concourse.bass_utils import run_bass_kernel_spmd

B, S, D, H = 2, 2048, 1024, 16
HD = 64          # head dim
HPC = 4          # heads per core
NCORES = 8
P = 128
NS = S // P      # 16 s-blocks
KC = D // P      # 8 d-chunks
CH = 512         # query-chunk width
NT = S // CH     # 4 query chunks
F32 = mybir.dt.float32
F16 = mybir.dt.float16
SCALE = 1.0 / np.sqrt(HD)
EXPB = -3.25     # exp(s*SCALE + EXPB): softmax-invariant shift; true max
                 # scaled score is ~7.95, keeps p well inside fp16 range
MASKC = 235.0    # per-step causal mask decrement; one step already zeroes exp

PSUM = bass.MemorySpace.PSUM


def _build_body(ctx: ExitStack, tc: "tile.TileContext", xt_d, wv_d, w16_d, o_d):
    nc = tc.nc

    persist = ctx.enter_context(tc.tile_pool(name="persist", bufs=1))
    ident_h = persist.tile([P, P], F16)
    masks.make_identity(nc, ident_h[:])

    # per-partition bias AP for exp(s*SCALE + EXPB)
    expb = persist.tile([P, 1], F32)
    nc.gpsimd.memset(expb[:], EXPB)

    # Causal-mask matmul operands: A[r,j] = [r<=j] (stationary),
    # B[r,i] = -MASKC*[r>i] (moving); A^T @ B = -MASKC*max(0, j-i).
    maskA = persist.tile([P, P], F16)
    nc.gpsimd.memset(maskA[:], 1.0)
    nc.gpsimd.affine_select(
        out=maskA[:],
        in_=maskA[:],
        compare_op=mybir.AluOpType.is_ge,
        fill=0.0,
        base=0,
        channel_multiplier=-1,
        pattern=[[1, P]],
    )
    maskB = persist.tile([P, P], F16)
    nc.gpsimd.memset(maskB[:], -MASKC)
    nc.gpsimd.affine_select(
        out=maskB[:],
        in_=maskB[:],
        compare_op=mybir.AluOpType.is_ge,
        fill=0.0,
        base=-1,
        channel_multiplier=1,
        pattern=[[-1, P]],
    )

    # v in natural layout + ones column, per (j-block, head): [128, 65]
    vaug = persist.tile([P, NS * HPC * 65], F16)
    nc.vector.memset(
        vaug[:].rearrange("p (n c) -> p n c", c=65)[:, :, 64:65], 1.0
    )
    out_sb = persist.tile([P, NS * HPC * HD], F32)

    # ---- static SBUF inputs -------------------------------------------
    wp = ctx.enter_context(tc.tile_pool(name="w", bufs=1))
    xT = wp.tile([P, KC * S], F16)             # 32KB/part, all chunks
    wv_sb = wp.tile([P, KC * 256], F16)        # 4KB/part
    w16_sb = wp.tile([P, KC * 4 * P], F16)     # 8KB/part, q/k weights

    # All input DMAs upfront on the sync queue, interleaved per d-chunk so
    # the first projection matmul has w16[k0]+xT[k0] after ~2 transfers.
    for k in range(KC):
        nc.sync.dma_start(
            xT[:, k * S: k * S + CH], xt_d[k * P:(k + 1) * P, 0:CH]
        )
        nc.sync.dma_start(
            w16_sb[:, k * 4 * P:(k + 1) * 4 * P], w16_d[k * P:(k + 1) * P, :]
        )
    for t in range(1, NT):
        for k in range(KC):
            nc.sync.dma_start(
                xT[:, k * S + t * CH: k * S + (t + 1) * CH],
                xt_d[k * P:(k + 1) * P, t * CH:(t + 1) * CH],
            )
    for k in range(KC):
        nc.scalar.dma_start(
            wv_sb[:, k * 256:(k + 1) * 256], wv_d[k * P:(k + 1) * P, :]
        )

    # ---- pools ---------------------------------------------------------
    qkp = ctx.enter_context(tc.tile_pool(name="qk", bufs=1))
    qkT = qkp.tile([P, 4 * S], F16)      # m0,m1 = q(h01,h23); m2,m3 = k

    ps_st = ctx.enter_context(tc.tile_pool(name="ps_st", bufs=2, space=PSUM))
    ps_o = ctx.enter_context(tc.tile_pool(name="ps_o", bufs=2, space=PSUM))
    ps_pj = ctx.enter_context(tc.tile_pool(name="ps_pj", bufs=2, space=PSUM))
    pp16 = ctx.enter_context(tc.tile_pool(name="p16", bufs=3))
    osbp = ctx.enter_context(tc.tile_pool(name="osb", bufs=4))
    rcp = ctx.enter_context(tc.tile_pool(name="rcol", bufs=4))

    def projqk(m, t):
        """qkT[:, m*S + t-chunk] = w_qk[col-block m]^T @ x^T."""
        pj = ps_pj.tile([P, CH], F32, tag="pj")
        for k in range(KC):
            nc.tensor.matmul(
                pj[:],
                w16_sb[:, k * 4 * P + m * P: k * 4 * P + (m + 1) * P],
                xT[:, k * S + t * CH: k * S + (t + 1) * CH],
                start=(k == 0),
                stop=(k == KC - 1),
            )
        nc.vector.tensor_copy(
            qkT[:, m * S + t * CH: m * S + (t + 1) * CH], pj[:]
        )

    def vdirect(sb):
        """vaug[s-block sb] = x[sb] @ w_v (natural layout), all 4 heads."""
        pv = ps_pj.tile([P, CH], F32, tag="pj")
        for k in range(KC):
            nc.tensor.matmul(
                pv[:, 0:256],
                xT[:, k * S + sb * P: k * S + (sb + 1) * P],
                wv_sb[:, k * 256:(k + 1) * 256],
                start=(k == 0),
                stop=(k == KC - 1),
            )
        nc.vector.tensor_copy(
            vaug[:, sb * HPC * 65:(sb + 1) * HPC * 65]
            .rearrange("p (g c) -> p g c", c=65)[:, :, 0:64],
            pv[:, 0:256].rearrange("p (g c) -> p g c", c=64),
        )

    out_view = out_sb[:].rearrange("p (i g d) -> p i g d", g=HPC, d=HD)

    def finalize(h, t, po_h):
        """Transpose outT to natural layout, divide by denominator."""
        osb = osbp.tile([65, CH], F16, tag="osb")
        nc.vector.tensor_copy(osb[:], po_h[:])
        fin32 = ps_pj.tile([P, CH], F32, tag="pj")
        fin = fin32.bitcast(F16)[:, 0:CH]
        for b4 in range(4):
            nc.tensor.transpose(
                fin[:, b4 * P:b4 * P + 65],
                osb[:, b4 * P:(b4 + 1) * P],
                ident_h[0:65, 0:65],
            )
        fin_view = fin[:, 0:CH].rearrange("p (n c) -> p n c", c=P)
        rc = rcp.tile([P, 4], F32, tag="rc")
        nc.vector.reciprocal(rc[:], fin_view[:, :, 64])
        nc.vector.tensor_mul(
            out_view[:, 4 * t:4 * t + 4, h, :],
            fin_view[:, :, 0:64],
            rc[:].broadcast_to([P, 4, HD]),
        )

    def attn(pair, t):
        """Heads 2*pair, 2*pair+1; query chunk t (i in [512t, 512t+512))."""
        hA, hB = 2 * pair, 2 * pair + 1
        qm, km = pair, 2 + pair
        njb = 4 * t + 4
        po = {hA: ps_o.tile([65, CH], F32, tag="o", name="po_a"),
              hB: ps_o.tile([65, CH], F32, tag="o", name="po_b")}
        for jb in range(njb):
            doff = jb - 4 * t
            off = P * doff if doff > 0 else 0
            diag = doff >= 0
            st = ps_st.tile([P, 1024], F32, tag="st")
            for hi, h in enumerate((hA, hB)):
                hb = (h % 2) * 64
                nc.tensor.matmul(
                    st[:, hi * CH + off:(hi + 1) * CH],
                    qkT[hb:hb + 64, km * S + jb * P: km * S + (jb + 1) * P],
                    qkT[hb:hb + 64, qm * S + t * CH + off: qm * S + (t + 1) * CH],
                    start=True,
                    stop=not diag,
                    tile_position=(hb, 0),
                )
                if diag:
                    c0 = hi * CH + off
                    nc.tensor.matmul(
                        st[:, c0:c0 + P], maskA[:], maskB[:],
                        start=False, stop=True,
                    )
            p16 = pp16.tile([P, 1024], F16, tag="p16")
            if doff < 2:
                # full-width exp; for doff in {0,1} the stale sub-window
                # region of st is never read by the trimmed AV below
                nc.scalar.activation(
                    p16[:], st[:], mybir.ActivationFunctionType.Exp,
                    scale=float(SCALE), bias=expb[:],
                )
            else:
                for hi in range(2):
                    nc.scalar.activation(
                        p16[:, hi * CH + off:(hi + 1) * CH],
                        st[:, hi * CH + off:(hi + 1) * CH],
                        mybir.ActivationFunctionType.Exp,
                        scale=float(SCALE), bias=expb[:],
                    )
            for hi, h in enumerate((hA, hB)):
                nc.tensor.matmul(
                    po[h][:, off:CH],
                    vaug[:, (jb * HPC + h) * 65:(jb * HPC + h + 1) * 65],
                    p16[:, hi * CH + off:(hi + 1) * CH],
                    start=(jb == 0),
                    stop=(jb == njb - 1),
                )
        for h in (hA, hB):
            finalize(h, t, po[h])

    # ---- main loop -----------------------------------------------------
    for t in range(NT):
        for m in (0, 2, 1, 3):
            projqk(m, t)
        for sb in range(4 * t, 4 * t + 4):
            vdirect(sb)
        attn(0, t)
        attn(1, t)
        for b4 in range(4):
            ib = 4 * t + b4
            nc.sync.dma_start(
                o_d[ib * P:(ib + 1) * P, :],
                out_sb[:, ib * HPC * HD:(ib + 1) * HPC * HD],
            )


def build_program():
    nc = bacc.Bacc(
        "TRN2",
        target_bir_lowering=False,
        debug=False,
        enable_asserts=True,
    )
    xt_d = nc.dram_tensor("xT", [D, S], F16, kind="ExternalInput").ap()
    wv_d = nc.dram_tensor("wv", [D, 256], F16, kind="ExternalInput").ap()
    w16_d = nc.dram_tensor("wqk16", [D, 4 * P], F16, kind="ExternalInput").ap()
    o_d = nc.dram_tensor("o", [S, HPC * HD], F32, kind="ExternalOutput").ap()

    with tile.TileContext(nc) as tc, ExitStack() as ctx:
        _build_body(ctx, tc, xt_d, wv_d, w16_d, o_d)
    nc.compile()
    return nc


_CACHE = {}


def _compiled():
    if "nc" not in _CACHE:
        _CACHE["nc"] = build_program()
    return _CACHE["nc"]


def make_in_maps(x, w_qkv):
    x = np.asarray(x, dtype=np.float32)
    w_qkv = np.asarray(w_qkv, dtype=np.float32)
    xT16 = [np.ascontiguousarray(x[b].T).astype(np.float16) for b in range(B)]
    in_maps = []
    for c in range(NCORES):
        b = c // 4
        cs = (c % 4) * HPC * HD
        wqk = np.concatenate(
            [w_qkv[:, cs:cs + 256], w_qkv[:, D + cs:D + cs + 256]], axis=1
        )
        wv = np.ascontiguousarray(w_qkv[:, 2 * D + cs:2 * D + cs + 256]).astype(
            np.float16
        )
        in_maps.append(
            {"xT": xT16[b], "wv": wv, "wqk16": wqk.astype(np.float16)}
        )
    return in_maps


def gather_out(results):
    out = np.empty((B, S, D), np.float32)
    for c in range(NCORES):
        b = c // 4
        cs = (c % 4) * HPC * HD
        out[b][:, cs:cs + HPC * HD] = results[c]["o"]
    return out


def kernel(x, w_qkv, w_o=None, **_):
    nc = _compiled()
    res = run_bass_kernel_spmd(nc, make_in_maps(x, w_qkv), core_ids=list(range(NCORES)))
    return gather_out(res.results)


# revision 31
# speedup vs baseline: 1.1007x; 1.0828x over previous
"""Trainium2 Bass kernel for causal self-attention (B=2, S=2048, D=1024, H=16).

Sharding: 8 cores = 2 batches x 4 head-groups. Core c handles batch c//4 and
heads 4*(c%4) .. 4*(c%4)+4. No cross-core communication; the host gathers the
output slices. w_o is unused by the reference.

Per-core kernel (Tile framework), fp16 matmul path with fp32 psum/softmax:
  1. All xT chunks + q/k weights are DMA'd upfront, interleaved per d-chunk
     on the sync queue so the first projection matmul starts ~1.5us in; wv
     goes on the scalar queue. Output DMAs are emitted per i-block but sit
     behind the (already drained) input DMAs, so they never stall the next
     chunk's projection.
  2. Projection with w stationary produces qT/kT [cols, s]; v is produced in
     natural [s, hd] layout (xT stationary) and augmented with a ones column
     (vaug) so the AV matmul also emits softmax denominators.
  3. Scores ST[j,i] = k_j . q_i per j-block with two heads packed into PE
     rows 0-63 / 64-127, trimmed to the causal window. Triangular masking of
     the diagonal 128x128 block happens inside the same PSUM accumulation
     group: one extra matmul adds A^T @ B = -235*max(0, j-i) (A[r,j]=[r<=j]
     stationary, B[r,i]=-235*[r>i] moving), so exp yields exact zeros and no
     other engine sits on the ST->exp critical path.
  4. exp on ACT (bias -3.25, softmax-invariant; true max scaled score ~7.95)
     over both heads in one instruction, split per head only for deep
     diagonal blocks where trimming saves real elements. AV accumulates
     outT[65, i] with v||ones stationary over the trimmed causal column
     range; a final PE transpose + reciprocal*mul yields natural layout.
"""

import sys

sys.path.insert(0, "/opt/trn_rl_repo")

from contextlib import ExitStack

import numpy as np

import concourse.bass as bass
import concourse.tile as tile
from concourse import bacc, masks, mybir
from concourse.bass_utils import run_bass_kernel_spmd

B, S, D, H = 2, 2048, 1024, 16
HD = 64          # head dim
HPC = 4          # heads per core
NCORES = 8
P = 128
NS = S // P      # 16 s-blocks
KC = D // P      # 8 d-chunks
CH = 512         # query-chunk width
NT = S // CH     # 4 query chunks
F32 = mybir.dt.float32
F16 = mybir.dt.float16
SCALE = 1.0 / np.sqrt(HD)
EXPB = -3.25     # exp(s*SCALE + EXPB): softmax-invariant shift; true max
                 # scaled score is ~7.95, keeps p well inside fp16 range
MASKC = 235.0    # per-step causal mask decrement; one step already zeroes exp

PSUM = bass.MemorySpace.PSUM


def _build_body(ctx: ExitStack, tc: "tile.TileContext", xt_d, wv_d, w16_d, o_d):
    nc = tc.nc

    persist = ctx.enter_context(tc.tile_pool(name="persist", bufs=1))
    ident_h = persist.tile([P, P], F16)
    masks.make_identity(nc, ident_h[:])

    # per-partition bias AP for exp(s*SCALE + EXPB)
    expb = persist.tile([P, 1], F32)
    nc.gpsimd.memset(expb[:], EXPB)

    # Causal-mask matmul operands: A[r,j] = [r<=j] (stationary),
    # B[r,i] = -MASKC*[r>i] (moving); A^T @ B = -MASKC*max(0, j-i).
    maskA = persist.tile([P, P], F16)
    nc.gpsimd.memset(maskA[:], 1.0)
    nc.gpsimd.affine_select(
        out=maskA[:],
        in_=maskA[:],
        compare_op=mybir.AluOpType.is_ge,
        fill=0.0,
        base=0,
        channel_multiplier=-1,
        pattern=[[1, P]],
    )
    maskB = persist.tile([P, P], F16)
    nc.gpsimd.memset(maskB[:], -MASKC)
    nc.gpsimd.affine_select(
        out=maskB[:],
        in_=maskB[:],
        compare_op=mybir.AluOpType.is_ge,
        fill=0.0,
        base=-1,
        channel_multiplier=1,
        pattern=[[-1, P]],
    )

    # v in natural layout + ones column, per (j-block, head): [128, 65]
    vaug = persist.tile([P, NS * HPC * 65], F16)
    nc.vector.memset(
        vaug[:].rearrange("p (n c) -> p n c", c=65)[:, :, 64:65], 1.0
    )
    out_sb = persist.tile([P, NS * HPC * HD], F32)

    # ---- static SBUF inputs -------------------------------------------
    wp = ctx.enter_context(tc.tile_pool(name="w", bufs=1))
    xT = wp.tile([P, KC * S], F16)             # 32KB/part, all chunks
    wv_sb = wp.tile([P, KC * 256], F16)        # 4KB/part
    w16_sb = wp.tile([P, KC * 4 * P], F16)     # 8KB/part, q/k weights

    # All input DMAs upfront, split across the sync and scalar queues so the
    # DMA-fed first projection drains two chunks per ~650ns instead of one:
    # sync carries xT(0), scalar carries w16+wv interleaved.
    for k in range(KC):
        nc.sync.dma_start(
            xT[:, k * S: k * S + CH], xt_d[k * P:(k + 1) * P, 0:CH]
        )
        nc.scalar.dma_start(
            w16_sb[:, k * 4 * P:(k + 1) * 4 * P], w16_d[k * P:(k + 1) * P, :]
        )
        nc.scalar.dma_start(
            wv_sb[:, k * 256:(k + 1) * 256], wv_d[k * P:(k + 1) * P, :]
        )
    for t in range(1, NT):
        for k in range(KC):
            nc.sync.dma_start(
                xT[:, k * S + t * CH: k * S + (t + 1) * CH],
                xt_d[k * P:(k + 1) * P, t * CH:(t + 1) * CH],
            )

    # ---- pools ---------------------------------------------------------
    qkp = ctx.enter_context(tc.tile_pool(name="qk", bufs=1))
    qkT = qkp.tile([P, 4 * S], F16)      # m0,m1 = q(h01,h23); m2,m3 = k

    # PSUM: st 2x2 banks + o 2x1 + pj 1 + fin 1 = 8 banks. fin is separate
    # from pj so the next chunk's projection never WAR-chains behind the
    # previous chunk's finalize (that serialized proj after attention).
    ps_st = ctx.enter_context(tc.tile_pool(name="ps_st", bufs=2, space=PSUM))
    ps_o = ctx.enter_context(tc.tile_pool(name="ps_o", bufs=2, space=PSUM))
    ps_pj = ctx.enter_context(tc.tile_pool(name="ps_pj", bufs=1, space=PSUM))
    ps_fin = ctx.enter_context(tc.tile_pool(name="ps_fin", bufs=1, space=PSUM))
    pp16 = ctx.enter_context(tc.tile_pool(name="p16", bufs=6))
    osbp = ctx.enter_context(tc.tile_pool(name="osb", bufs=4))
    rcp = ctx.enter_context(tc.tile_pool(name="rcol", bufs=4))

    def projqk(m, t):
        """qkT[:, m*S + t-chunk] = w_qk[col-block m]^T @ x^T."""
        pj = ps_pj.tile([P, CH], F32, tag="pj")
        for k in range(KC):
            nc.tensor.matmul(
                pj[:],
                w16_sb[:, k * 4 * P + m * P: k * 4 * P + (m + 1) * P],
                xT[:, k * S + t * CH: k * S + (t + 1) * CH],
                start=(k == 0),
                stop=(k == KC - 1),
            )
        nc.vector.tensor_copy(
            qkT[:, m * S + t * CH: m * S + (t + 1) * CH], pj[:]
        )

    def vdirect(sb):
        """vaug[s-block sb] = x[sb] @ w_v (natural layout), all 4 heads."""
        pv = ps_pj.tile([P, CH], F32, tag="pj")
        for k in range(KC):
            nc.tensor.matmul(
                pv[:, 0:256],
                xT[:, k * S + sb * P: k * S + (sb + 1) * P],
                wv_sb[:, k * 256:(k + 1) * 256],
                start=(k == 0),
                stop=(k == KC - 1),
            )
        nc.vector.tensor_copy(
            vaug[:, sb * HPC * 65:(sb + 1) * HPC * 65]
            .rearrange("p (g c) -> p g c", c=65)[:, :, 0:64],
            pv[:, 0:256].rearrange("p (g c) -> p g c", c=64),
        )

    out_view = out_sb[:].rearrange("p (i g d) -> p i g d", g=HPC, d=HD)

    def finalize(h, t, po_h):
        """Transpose outT to natural layout, divide by denominator."""
        osb = osbp.tile([65, CH], F16, tag="osb")
        nc.vector.tensor_copy(osb[:], po_h[:])
        fin32 = ps_fin.tile([P, CH], F32, tag="fin")
        fin = fin32.bitcast(F16)[:, 0:CH]
        for b4 in range(4):
            nc.tensor.transpose(
                fin[:, b4 * P:b4 * P + 65],
                osb[:, b4 * P:(b4 + 1) * P],
                ident_h[0:65, 0:65],
            )
        fin_view = fin[:, 0:CH].rearrange("p (n c) -> p n c", c=P)
        rc = rcp.tile([P, 4], F32, tag="rc")
        nc.vector.reciprocal(rc[:], fin_view[:, :, 64])
        nc.vector.tensor_mul(
            out_view[:, 4 * t:4 * t + 4, h, :],
            fin_view[:, :, 0:64],
            rc[:].broadcast_to([P, 4, HD]),
        )

    def attn(pair, t):
        """Heads 2*pair, 2*pair+1; query chunk t (i in [512t, 512t+512))."""
        hA, hB = 2 * pair, 2 * pair + 1
        qm, km = pair, 2 + pair
        njb = 4 * t + 4
        po = {hA: ps_o.tile([65, CH], F32, tag="o", name="po_a"),
              hB: ps_o.tile([65, CH], F32, tag="o", name="po_b")}
        for jb in range(njb):
            doff = jb - 4 * t
            off = P * doff if doff > 0 else 0
            diag = doff >= 0
            st = ps_st.tile([P, 1024], F32, tag="st")
            for hi, h in enumerate((hA, hB)):
                hb = (h % 2) * 64
                nc.tensor.matmul(
                    st[:, hi * CH + off:(hi + 1) * CH],
                    qkT[hb:hb + 64, km * S + jb * P: km * S + (jb + 1) * P],
                    qkT[hb:hb + 64, qm * S + t * CH + off: qm * S + (t + 1) * CH],
                    start=True,
                    stop=not diag,
                    tile_position=(hb, 0),
                )
                if diag:
                    c0 = hi * CH + off
                    nc.tensor.matmul(
                        st[:, c0:c0 + P], maskA[:], maskB[:],
                        start=False, stop=True,
                    )
            p16 = pp16.tile([P, 1024], F16, tag="p16")
            if doff < 2:
                # full-width exp; for doff in {0,1} the stale sub-window
                # region of st is never read by the trimmed AV below
                nc.scalar.activation(
                    p16[:], st[:], mybir.ActivationFunctionType.Exp,
                    scale=float(SCALE), bias=expb[:],
                )
            else:
                for hi in range(2):
                    nc.scalar.activation(
                        p16[:, hi * CH + off:(hi + 1) * CH],
                        st[:, hi * CH + off:(hi + 1) * CH],
                        mybir.ActivationFunctionType.Exp,
                        scale=float(SCALE), bias=expb[:],
                    )
            for hi, h in enumerate((hA, hB)):
                nc.tensor.matmul(
                    po[h][:, off:CH],
                    vaug[:, (jb * HPC + h) * 65:(jb * HPC + h + 1) * 65],
                    p16[:, hi * CH + off:(hi + 1) * CH],
                    start=(jb == 0),
                    stop=(jb == njb - 1),
                )
        for h in (hA, hB):
            finalize(h, t, po[h])

    # ---- main loop -----------------------------------------------------
    for t in range(NT):
        for m in (0, 2, 1, 3):
            projqk(m, t)
        for sb in range(4 * t, 4 * t + 4):
            vdirect(sb)
        attn(0, t)
        attn(1, t)
        for b4 in range(4):
            ib = 4 * t + b4
            nc.sync.dma_start(
                o_d[ib * P:(ib + 1) * P, :],
                out_sb[:, ib * HPC * HD:(ib + 1) * HPC * HD],
            )


def build_program():
    nc = bacc.Bacc(
        "TRN2",
        target_bir_lowering=False,
        debug=False,
        enable_asserts=True,
    )
    xt_d = nc.dram_tensor("xT", [D, S], F16, kind="ExternalInput").ap()
    wv_d = nc.dram_tensor("wv", [D, 256], F16, kind="ExternalInput").ap()
    w16_d = nc.dram_tensor("wqk16", [D, 4 * P], F16, kind="ExternalInput").ap()
    o_d = nc.dram_tensor("o", [S, HPC * HD], F32, kind="ExternalOutput").ap()

    with tile.TileContext(nc) as tc, ExitStack() as ctx:
        _build_body(ctx, tc, xt_d, wv_d, w16_d, o_d)
    nc.compile()
    return nc


_CACHE = {}


def _compiled():
    if "nc" not in _CACHE:
        _CACHE["nc"] = build_program()
    return _CACHE["nc"]


def make_in_maps(x, w_qkv):
    x = np.asarray(x, dtype=np.float32)
    w_qkv = np.asarray(w_qkv, dtype=np.float32)
    xT16 = [np.ascontiguousarray(x[b].T).astype(np.float16) for b in range(B)]
    in_maps = []
    for c in range(NCORES):
        b = c // 4
        cs = (c % 4) * HPC * HD
        wqk = np.concatenate(
            [w_qkv[:, cs:cs + 256], w_qkv[:, D + cs:D + cs + 256]], axis=1
        )
        wv = np.ascontiguousarray(w_qkv[:, 2 * D + cs:2 * D + cs + 256]).astype(
            np.float16
        )
        in_maps.append(
            {"xT": xT16[b], "wv": wv, "wqk16": wqk.astype(np.float16)}
        )
    return in_maps


def gather_out(results):
    out = np.empty((B, S, D), np.float32)
    for c in range(NCORES):
        b = c // 4
        cs = (c % 4) * HPC * HD
        out[b][:, cs:cs + HPC * HD] = results[c]["o"]
    return out


def kernel(x, w_qkv, w_o=None, **_):
    nc = _compiled()
    res = run_bass_kernel_spmd(nc, make_in_maps(x, w_qkv), core_ids=list(range(NCORES)))
    return gather_out(res.results)
